# revision 9
# baseline (speedup 1.0000x reference)
"""Trainium2 Bass kernel for a single transformer decoder layer.

Strategy (8 NeuronCores):
- Attention: tensor-parallel over heads. Core c owns heads {2c, 2c+1} =
  model dims [128c, 128c+128). Each core computes Q/K/V projections for
  its 128 dims over all tokens, causal attention for its 2 heads, and the
  partial W_o contribution. Partials are combined with a chunked
  ReduceScatter that hands core c the 512 tokens [512c, 512c+512).
- FFN: token-parallel. Each core runs LN1 + the full FFN (d_ff=4096) +
  LN2 for its 512 tokens. No second collective; host concatenates the 8
  token slices.
- Matmuls run as float32r (full-rate fp32 storage, reduced mantissa in
  the PE), everything else fp32.
"""
import os
import sys

for _p in ("/opt/trn_rl_repo", "/root/.axon_site/_ro/trn_rl_repo"):
    if os.path.isdir(_p) and _p not in sys.path:
        sys.path.insert(0, _p)

import numpy as np
from concourse import bacc, mybir
from concourse.bass_utils import run_bass_kernel_spmd
from concourse.tile import TileContext

F32 = mybir.dt.float32
F32R = mybir.dt.float32r

B = 2
T = 2048
D = 1024
D_FF = 4096
NCORES = 8
TPC = (B * T) // NCORES      # tokens per core for the FFN = 512
NEG = -1e9
EPS = 1e-5

KT = 8            # k-tiles over D (contraction 1024)
QC = 4            # q chunks of 512 per sequence
TT = T // 128     # 16 token tiles per sequence
FT = D_FF // 128  # 32 f-tiles
NTT = TPC // 128  # 4 token tiles per core in FFN phase


def r(ap):
    return ap.bitcast(F32R)


def build_kernel(n_rep=1):
    nc = bacc.Bacc("TRN2", target_bir_lowering=False, debug=False)
    nc.num_devices = NCORES
    AF = mybir.ActivationFunctionType

    # ---- parameters ----
    xt = nc.declare_dram_parameter("xt", [B, 128, KT, T], F32R, isOutput=False)
    xfm = nc.declare_dram_parameter("xfm", [TPC, D], F32, isOutput=False)
    wq = nc.declare_dram_parameter("wq", [128, KT, 128], F32R, isOutput=False)
    wk = nc.declare_dram_parameter("wk", [128, KT, 128], F32R, isOutput=False)
    wv = nc.declare_dram_parameter("wv", [128, KT, 128], F32R, isOutput=False)
    bq = nc.declare_dram_parameter("bq", [128, 1], F32, isOutput=False)
    bk = nc.declare_dram_parameter("bk", [128, 1], F32, isOutput=False)
    bv = nc.declare_dram_parameter("bv", [128, 1], F32, isOutput=False)
    wo = nc.declare_dram_parameter("wo", [128, D], F32R, isOutput=False)
    w1 = nc.declare_dram_parameter("w1", [128, KT, D_FF], F32R, isOutput=False)
    b1s = nc.declare_dram_parameter("b1s", [128, FT], F32, isOutput=False)
    w2 = nc.declare_dram_parameter("w2", [128, FT, D], F32R, isOutput=False)
    b2b = nc.declare_dram_parameter("b2b", [128, D], F32, isOutput=False)
    bob = nc.declare_dram_parameter("bob", [128, D], F32, isOutput=False)
    l1g = nc.declare_dram_parameter("l1g", [128, D], F32, isOutput=False)
    l1b = nc.declare_dram_parameter("l1b", [128, D], F32, isOutput=False)
    l2g = nc.declare_dram_parameter("l2g", [128, D], F32, isOutput=False)
    l2b = nc.declare_dram_parameter("l2b", [128, D], F32, isOutput=False)
    msk = nc.declare_dram_parameter("msk", [128, 4, 512], F32, isOutput=False)
    idt = nc.declare_dram_parameter("idt", [128, 128], F32R, isOutput=False)
    on64 = nc.declare_dram_parameter("on64", [1, 64], F32R, isOutput=False)
    on128 = nc.declare_dram_parameter("on128", [128, 1], F32R, isOutput=False)
    out = nc.declare_dram_parameter("out", [TPC, D], F32, isOutput=True)

    # internal DRAM
    apart = [nc.dram_tensor(f"apart{rep}", [NTT, NCORES * 128, D], F32)
             for rep in range(n_rep)]
    rso = [nc.dram_tensor(f"rso{rep}", [NTT, 128, D], F32) for rep in range(n_rep)]

    with TileContext(nc) as tc:
        with tc.tile_pool(name="poolA", bufs=1) as pA, \
             tc.tile_pool(name="psum", bufs=1, space="PSUM") as pp:

            # ---- persistent small tiles ----
            wq_s = pA.tile([128, KT, 128], F32R, tag="wq")
            wk_s = pA.tile([128, KT, 128], F32R, tag="wk")
            wv_s = pA.tile([128, KT, 128], F32R, tag="wv")
            bq_s = pA.tile([128, 1], F32, tag="bq")
            bk_s = pA.tile([128, 1], F32, tag="bk")
            bv_s = pA.tile([128, 1], F32, tag="bv")
            wo_s = pA.tile([128, D], F32R, tag="wo")
            b1_s = pA.tile([128, FT], F32, tag="b1")
            b2_s = pA.tile([128, D], F32, tag="b2")
            bo_s = pA.tile([128, D], F32, tag="bo")
            l1g_s = pA.tile([128, D], F32, tag="l1g")
            l1b_s = pA.tile([128, D], F32, tag="l1b")
            l2g_s = pA.tile([128, D], F32, tag="l2g")
            l2b_s = pA.tile([128, D], F32, tag="l2b")
            msk_s = pA.tile([128, 4, 512], F32, tag="msk")
            idt_s = pA.tile([128, 128], F32R, tag="idt")
            on_s = pA.tile([1, 64], F32R, tag="on64")
            on1_s = pA.tile([128, 1], F32R, tag="on128")
            eps_s = pA.tile([128, 1], F32, tag="eps")
            nc.vector.memset(eps_s[:, :], EPS)
            for dst, src in [(wq_s, wq), (wk_s, wk), (wv_s, wv),
                             (bq_s, bq), (bk_s, bk), (bv_s, bv), (wo_s, wo),
                             (b1_s, b1s), (b2_s, b2b), (bo_s, bob),
                             (l1g_s, l1g), (l1b_s, l1b), (l2g_s, l2g),
                             (l2b_s, l2b), (idt_s, idt), (on_s, on64), (on1_s, on128)]:
                nc.sync.dma_start(out=dst[...], in_=src[...])
            nc.sync.dma_start(out=msk_s[:, :, :], in_=msk[:, :, :])

            for rep in range(n_rep):
                emit_once(nc, tc, pA, pp, rep, apart[rep], rso[rep],
                          xt, xfm, out,
                          wq_s, wk_s, wv_s, bq_s, bk_s, bv_s, wo_s,
                          w1, b1_s, w2, b2_s, bo_s,
                          l1g_s, l1b_s, l2g_s, l2b_s, msk_s, idt_s, on_s,
                          on1_s, eps_s, AF)
    nc.compile()
    return nc


def layer_norm_inplace(nc, pC, x_t, g_s, b_s, eps_s, AF, tag, final_f32r=False):
    """LN over the free dim (1024) of x_t (128, 1024), in place."""
    st = pC.tile([128, 1], F32, tag=f"st{tag}", name=f"st{tag}")
    nc.vector.tensor_reduce(st[:, :], x_t[:, :], mybir.AxisListType.X,
                            mybir.AluOpType.add)
    nmu = pC.tile([128, 1], F32, tag=f"nmu{tag}", name=f"nmu{tag}")
    nc.scalar.activation(nmu[:, :], st[:, :], AF.Copy, scale=-1.0 / D)
    nc.vector.tensor_scalar_add(x_t[:, :], x_t[:, :], nmu[:, :])
    sq = pC.tile([128, D], F32, tag=f"sq{tag}", name=f"sq{tag}")
    vs = pC.tile([128, 1], F32, tag=f"vs{tag}", name=f"vs{tag}")
    nc.scalar.activation(sq[:, :], x_t[:, :], AF.Square, accum_out=vs[:, :])
    sd = pC.tile([128, 1], F32, tag=f"sd{tag}", name=f"sd{tag}")
    nc.scalar.activation(sd[:, :], vs[:, :], AF.Sqrt, scale=1.0 / D,
                         bias=eps_s[:, :])
    rstd = pC.tile([128, 1], F32, tag=f"rstd{tag}", name=f"rstd{tag}")
    nc.vector.reciprocal(rstd[:, :], sd[:, :])
    nc.vector.tensor_scalar_mul(x_t[:, :], x_t[:, :], rstd[:, :])
    nc.vector.tensor_mul(out=x_t[:, :], in0=x_t[:, :], in1=g_s[:, :])
    last_out = r(x_t[:, :]) if final_f32r else x_t[:, :]
    nc.vector.tensor_add(out=last_out, in0=x_t[:, :], in1=b_s[:, :])


def emit_once(nc, tc, pA, pp, rep, apart, rso, xt, xfm, out,
              wq_s, wk_s, wv_s, bq_s, bk_s, bv_s, wo_s,
              w1, b1_s, w2, b2_s, bo_s,
              l1g_s, l1b_s, l2g_s, l2b_s, msk_s, idt_s, on_s, on1_s, eps_s, AF):
    RG = [list(range(NCORES))]

    with tc.tile_pool(name=f"poolB{rep}", bufs=1) as pB:
        # ctx2[b][qc]: (128 d, 512 q) normalized context, both heads stacked
        ctx2 = {}
        for b in range(B):
            for qc in range(QC):
                ctx2[(b, qc)] = pA.tile([128, 512], F32R, tag=f"c2_{b}_{qc}",
                                        name=f"c2_{b}_{qc}")

        for b in range(B):
            qt_s = pB.tile([128, T], F32R, tag="qt")
            kt_s = pB.tile([128, T], F32R, tag="kt")
            vt_s = pB.tile([128, T], F32R, tag="vt")
            # ---- QKV projections for this batch ----
            for tc_ in range(QC):
                xts = pB.tile([128, KT, 512], F32R, tag="xts", bufs=2)
                nc.sync.dma_start(out=xts[:, :, :],
                                  in_=xt[b, :, :, tc_ * 512:(tc_ + 1) * 512])
                for (w_s, b_s, dst) in ((wq_s, bq_s, qt_s), (wk_s, bk_s, kt_s),
                                        (wv_s, bv_s, vt_s)):
                    ps = pp.tile([128, 512], F32, tag="mm", bufs=2, name="ps")
                    for k in range(KT):
                        nc.tensor.matmul(ps[:, :], w_s[:, k, :], xts[:, k, :],
                                         start=(k == 0), stop=(k == KT - 1))
                    nc.scalar.activation(dst[:, tc_ * 512:(tc_ + 1) * 512],
                                         ps[:, :], AF.Identity, bias=b_s[:, :])

            # ---- V transpose: vT (128 dims, T) -> v2 tiles (128 tok, 130) ----
            v2 = []
            for kt in range(TT):
                tp = pp.tile([128, 128], F32R, tag="mm", bufs=2, name=f"tp_{kt}")
                nc.tensor.transpose(tp[:, :], vt_s[:, kt * 128:(kt + 1) * 128],
                                    idt_s[:, :])
                v2t = pB.tile([128, 130], F32R, tag=f"v2_{kt}", name=f"v2_{kt}")
                nc.vector.tensor_copy(v2t[:, 64:65], on1_s[:, :])
                nc.vector.tensor_copy(v2t[:, 129:130], on1_s[:, :])
                nc.vector.tensor_copy(v2t[:, 0:64], tp[:, 0:64])
                nc.vector.tensor_copy(v2t[:, 65:129], tp[:, 64:128])
                v2.append(v2t)

            # ---- causal attention, per head ----
            for h in range(2):
                hs = slice(h * 64, (h + 1) * 64)
                vs_lo, vs_hi = (0, 65) if h == 0 else (65, 130)
                for qc in range(QC):
                    ctxp = pp.tile([65, 512], F32, tag="ctx", bufs=2, name="ctxp")
                    n_kt = 4 * qc + 4
                    prev_p = None
                    for kt in range(n_kt):
                        sp = pp.tile([128, 512], F32, tag="mm", bufs=2, name="sp")
                        nc.tensor.matmul(
                            sp[:, :],
                            kt_s[hs, kt * 128:(kt + 1) * 128],
                            qt_s[hs, qc * 512:(qc + 1) * 512],
                            start=True, stop=True)
                        if kt >= 4 * qc:
                            nc.vector.tensor_add(out=sp[:, :], in0=sp[:, :],
                                                 in1=msk_s[:, kt % 4, :])
                        p = pB.tile([128, 512], F32R, tag="p", bufs=4, name="p")
                        nc.scalar.activation(p[:, :], sp[:, :], AF.Exp,
                                             scale=0.125)
                        if prev_p is not None:
                            pkt, pt_ = prev_p
                            nc.tensor.matmul(ctxp[:, :], v2[pkt][:, vs_lo:vs_hi],
                                             pt_[:, :],
                                             start=(pkt == 0), stop=False)
                        prev_p = (kt, p)
                    pkt, pt_ = prev_p
                    nc.tensor.matmul(ctxp[:, :], v2[pkt][:, vs_lo:vs_hi],
                                     pt_[:, :], start=(pkt == 0), stop=True)
                    # normalize: rows 0..63 ctx, row 64 denominator
                    rec = pB.tile([1, 512], F32R, tag="rec", bufs=2, name="rec")
                    with nc.allow_low_precision(reason="f32r is full precision"):
                        nc.vector.reciprocal(rec[:, :], ctxp[64:65, :])
                    bc = pp.tile([64, 512], F32, tag="mm", bufs=2, name="bc")
                    nc.tensor.matmul(bc[:, :], on_s[:, :], rec[:, :],
                                     start=True, stop=True)
                    bcs = pB.tile([64, 512], F32, tag="bcs", bufs=2, name="bcs")
                    nc.scalar.activation(bcs[:, :], bc[:, :], AF.Copy)
                    nc.vector.tensor_mul(out=ctx2[(b, qc)][hs.start:hs.stop, :],
                                         in0=ctxp[0:64, :], in1=bcs[:, :])

        # ---- W_o partials + chunked ReduceScatter ----
        for tt in range(NTT):
            for b in range(B):
                for qc in range(QC):
                    ps = pp.tile([128, D], F32, tag="f2a", bufs=1,
                                 name=f"wops_{tt}_{b}_{qc}")
                    for oc in range(2):
                        nc.tensor.matmul(
                            ps[:, oc * 512:(oc + 1) * 512],
                            ctx2[(b, qc)][:, tt * 128:(tt + 1) * 128],
                            wo_s[:, oc * 512:(oc + 1) * 512],
                            start=True, stop=True)
                    ao = pB.tile([128, D], F32, tag="ao", bufs=3, name="ao")
                    nc.scalar.activation(ao[:, :], ps[:, :], AF.Copy)
                    core = 4 * b + qc
                    nc.sync.dma_start(
                        out=apart[tt, core * 128:(core + 1) * 128, :],
                        in_=ao[:, :])
            nc.gpsimd.collective_compute(
                "ReduceScatter", mybir.AluOpType.add, replica_groups=RG,
                ins=[apart[tt]], outs=[rso[tt]])

    # ---- FFN phase ----
    with tc.tile_pool(name=f"poolC{rep}", bufs=1) as pC:
        x1t_s = pC.tile([128, KT, TPC], F32R, tag="x1t")
        x1 = []
        for tt in range(NTT):
            xr = pC.tile([128, D], F32, tag="xr", bufs=1, name="xr")
            nc.sync.dma_start(out=xr[:, :], in_=xfm[tt * 128:(tt + 1) * 128, :])
            asb = pC.tile([128, D], F32, tag="asb", bufs=1, name="asb")
            nc.sync.dma_start(out=asb[:, :], in_=rso[tt, :, :])
            x1_tt = pA.tile([128, D], F32, tag=f"x1_{tt}", name=f"x1_{tt}")
            nc.vector.tensor_add(out=x1_tt[:, :], in0=asb[:, :], in1=bo_s[:, :])
            nc.vector.tensor_add(out=x1_tt[:, :], in0=x1_tt[:, :], in1=xr[:, :])
            layer_norm_inplace(nc, pC, x1_tt, l1g_s, l1b_s, eps_s, AF, "a")
            x1.append(x1_tt)
            x1c = pC.tile([128, D], F32R, tag="x1c", bufs=2, name="x1c")
            nc.vector.tensor_copy(x1c[:, :], x1_tt[:, :])
            for i in range(KT):
                tp2 = pp.tile([128, 128], F32R, tag="mm", bufs=2,
                              name=f"tp2_{tt}_{i}")
                nc.tensor.transpose(tp2[:, :], x1c[:, i * 128:(i + 1) * 128],
                                    idt_s[:, :])
                nc.vector.tensor_copy(x1t_s[:, i, tt * 128:(tt + 1) * 128],
                                      tp2[:, :])

        # FFN1: hT[ft] (128 f, 512 t)
        hts = []
        for ft in range(FT):
            w1s = pC.tile([128, KT, 128], F32R, tag="w1s", bufs=2, name="w1s")
            nc.sync.dma_start(out=w1s[:, :, :],
                              in_=w1[:, :, ft * 128:(ft + 1) * 128])
            ps = pp.tile([128, 512], F32, tag="mm", bufs=2, name=f"f1ps_{ft}")
            for k in range(KT):
                nc.tensor.matmul(ps[:, :], w1s[:, k, :], x1t_s[:, k, :],
                                 start=(k == 0), stop=(k == KT - 1))
            h_ft = pC.tile([128, TPC], F32R, tag=f"h_{ft}", name=f"h_{ft}")
            nc.scalar.activation(h_ft[:, :], ps[:, :], AF.Relu,
                                 bias=b1_s[:, ft:ft + 1])
            hts.append(h_ft)

        # FFN2 + LN2, token tiles in pairs
        for pair in range(NTT // 2):
            tts = (2 * pair, 2 * pair + 1)
            pss = {tts[0]: pp.tile([128, D], F32, tag="f2a", bufs=1,
                                   name=f"f2a_{pair}"),
                   tts[1]: pp.tile([128, D], F32, tag="f2b", bufs=1,
                                   name=f"f2b_{pair}")}
            for kf in range(FT):
                w2s = pC.tile([128, D], F32R, tag="w2s", bufs=2, name="w2s")
                nc.sync.dma_start(out=w2s[:, :], in_=w2[:, kf, :])
                for tt in tts:
                    for oc in range(2):
                        nc.tensor.matmul(
                            pss[tt][:, oc * 512:(oc + 1) * 512],
                            hts[kf][:, tt * 128:(tt + 1) * 128],
                            w2s[:, oc * 512:(oc + 1) * 512],
                            start=(kf == 0), stop=(kf == FT - 1))
            for tt in tts:
                y = pC.tile([128, D], F32, tag="y", bufs=1, name="y")
                nc.vector.tensor_add(out=y[:, :], in0=pss[tt][:, :],
                                     in1=b2_s[:, :])
                nc.vector.tensor_add(out=y[:, :], in0=y[:, :], in1=x1[tt][:, :])
                layer_norm_inplace(nc, pC, y, l2g_s, l2b_s, eps_s, AF, "b")
                nc.sync.dma_start(out=out[tt * 128:(tt + 1) * 128, :],
                                  in_=y[:, :])


# ---------------- host side ----------------

_CACHE = {}


def _get_kernel(n_rep=1):
    if n_rep not in _CACHE:
        _CACHE[n_rep] = build_kernel(n_rep)
    return _CACHE[n_rep]


def make_in_maps(x, Wq, bq, Wk, bk, Wv, bv, Wo, bo, W1, b1, W2, b2,
                 ln1_g, ln1_b, ln2_g, ln2_b):
    x = np.asarray(x, np.float32)
    xf = x.reshape(B * T, D)
    # xt[b, p, k, t] = x[b, t, 128k+p]
    xtv = np.ascontiguousarray(
        x.reshape(B, T, KT, 128).transpose(0, 3, 2, 1))
    w1t = np.ascontiguousarray(
        np.asarray(W1, np.float32).T.reshape(KT, 128, D_FF).transpose(1, 0, 2))
    w2t = np.ascontiguousarray(
        np.asarray(W2, np.float32).T.reshape(FT, 128, D).transpose(1, 0, 2))
    b1sv = np.ascontiguousarray(np.asarray(b1, np.float32).reshape(FT, 128).T)
    ones = np.ones((128, 1), np.float32)
    b2bv = ones @ np.asarray(b2, np.float32)[None, :]
    bobv = ones @ np.asarray(bo, np.float32)[None, :]
    l1gb = ones @ np.asarray(ln1_g, np.float32)[None, :]
    l1bb = ones @ np.asarray(ln1_b, np.float32)[None, :]
    l2gb = ones @ np.asarray(ln2_g, np.float32)[None, :]
    l2bb = ones @ np.asarray(ln2_b, np.float32)[None, :]
    # additive causal masks for diagonal blocks, r = kt%4; layout (128,4,512)
    kk = np.arange(128)[:, None]
    qq = np.arange(512)[None, :]
    mskv = np.ascontiguousarray(np.stack(
        [np.where(qq >= r * 128 + kk, 0.0, NEG).astype(np.float32)
         for r in range(4)]).transpose(1, 0, 2))
    idtv = np.eye(128, dtype=np.float32)
    on64v = np.ones((1, 64), np.float32)

    in_maps = []
    for c in range(NCORES):
        sl = slice(128 * c, 128 * (c + 1))

        def wslice(W):
            # [128 p(in-dim), KT, 128 m(out-dim)]: w[p,k,m]=W[128c+m, 128k+p]
            Ws = np.asarray(W, np.float32)[sl, :]  # (128 out, 1024 in)
            return np.ascontiguousarray(
                Ws.T.reshape(KT, 128, 128).transpose(1, 0, 2))

        m = dict(
            xt=xtv, xfm=np.ascontiguousarray(xf[c * TPC:(c + 1) * TPC]),
            wq=wslice(Wq), wk=wslice(Wk), wv=wslice(Wv),
            bq=np.asarray(bq, np.float32)[sl][:, None].copy(),
            bk=np.asarray(bk, np.float32)[sl][:, None].copy(),
            bv=np.asarray(bv, np.float32)[sl][:, None].copy(),
            wo=np.ascontiguousarray(np.asarray(Wo, np.float32)[:, sl].T),
            w1=w1t, b1s=b1sv, w2=w2t, b2b=b2bv, bob=bobv,
            l1g=l1gb, l1b=l1bb, l2g=l2gb, l2b=l2bb,
            msk=mskv, idt=idtv, on64=on64v, on128=ones,
        )
        in_maps.append(m)
    return in_maps


def kernel(x, mask, Wq, bq, Wk, bk, Wv, bv, Wo, bo, W1, b1, W2, b2,
           ln1_g, ln1_b, ln2_g, ln2_b):
    del mask  # causal mask is hardcoded in the device program
    nc = _get_kernel(1)
    in_maps = make_in_maps(x, Wq, bq, Wk, bk, Wv, bv, Wo, bo, W1, b1, W2, b2,
                           ln1_g, ln1_b, ln2_g, ln2_b)
    res = run_bass_kernel_spmd(nc, in_maps, list(range(NCORES)))
    full = np.concatenate([res.results[c]["out"] for c in range(NCORES)], axis=0)
    return full.reshape(B, T, D).astype(np.float32)


# revision 14
# speedup vs baseline: 5.4623x; 5.4623x over previous
"""Trainium2 Bass kernel for a single transformer decoder layer.

Strategy (8 NeuronCores):
- Attention: tensor-parallel over heads. Core c owns heads {2c, 2c+1} =
  model dims [128c, 128c+128). Each core computes Q/K/V projections for
  its 128 dims over all tokens, causal attention for its 2 heads, and the
  partial W_o contribution. Partials are combined with a chunked
  ReduceScatter that hands core c the 512 tokens [512c, 512c+512).
- FFN: token-parallel. Each core runs LN1 + the full FFN (d_ff=4096) +
  LN2 for its 512 tokens. No second collective; host concatenates the 8
  token slices.
- Matmuls run as float32r (full-rate fp32 storage, reduced mantissa in
  the PE), everything else fp32.
"""
import os
import sys

for _p in ("/opt/trn_rl_repo", "/root/.axon_site/_ro/trn_rl_repo"):
    if os.path.isdir(_p) and _p not in sys.path:
        sys.path.insert(0, _p)

import numpy as np
from concourse import bacc, mybir
from concourse.bass_utils import run_bass_kernel_spmd
from concourse.tile import TileContext

F32 = mybir.dt.float32
F32R = mybir.dt.float32r

B = 2
T = 2048
D = 1024
D_FF = 4096
NCORES = 8
TPC = (B * T) // NCORES      # tokens per core for the FFN = 512
NEG = -1e9
EPS = 1e-5

KT = 8            # k-tiles over D (contraction 1024)
QC = 4            # q chunks of 512 per sequence
TT = T // 128     # 16 token tiles per sequence
FT = D_FF // 128  # 32 f-tiles
NTT = TPC // 128  # 4 token tiles per core in FFN phase


def r(ap):
    return ap.bitcast(F32R)


def build_kernel(n_rep=1, skip=(), stop_after=None, attn_mode=None):
    nc = bacc.Bacc("TRN2", target_bir_lowering=False, debug=False)
    nc.num_devices = NCORES
    AF = mybir.ActivationFunctionType

    # ---- parameters ----
    xt = nc.declare_dram_parameter("xt", [B, 128, KT, T], F32R, isOutput=False)
    xfm = nc.declare_dram_parameter("xfm", [TPC, D], F32, isOutput=False)
    wq = nc.declare_dram_parameter("wq", [128, KT, 128], F32R, isOutput=False)
    wk = nc.declare_dram_parameter("wk", [128, KT, 128], F32R, isOutput=False)
    wv = nc.declare_dram_parameter("wv", [128, KT, 128], F32R, isOutput=False)
    bq = nc.declare_dram_parameter("bq", [128, 1], F32, isOutput=False)
    bk = nc.declare_dram_parameter("bk", [128, 1], F32, isOutput=False)
    bv = nc.declare_dram_parameter("bv", [128, 1], F32, isOutput=False)
    wo = nc.declare_dram_parameter("wo", [128, D], F32R, isOutput=False)
    w1 = nc.declare_dram_parameter("w1", [128, KT, D_FF], F32R, isOutput=False)
    b1s = nc.declare_dram_parameter("b1s", [128, FT], F32, isOutput=False)
    w2 = nc.declare_dram_parameter("w2", [128, FT, D], F32R, isOutput=False)
    b2b = nc.declare_dram_parameter("b2b", [128, D], F32, isOutput=False)
    bob = nc.declare_dram_parameter("bob", [128, D], F32, isOutput=False)
    l1g = nc.declare_dram_parameter("l1g", [128, D], F32, isOutput=False)
    l1b = nc.declare_dram_parameter("l1b", [128, D], F32, isOutput=False)
    l2g = nc.declare_dram_parameter("l2g", [128, D], F32, isOutput=False)
    l2b = nc.declare_dram_parameter("l2b", [128, D], F32, isOutput=False)
    msk = nc.declare_dram_parameter("msk", [128, 4, 512], F32, isOutput=False)
    idt = nc.declare_dram_parameter("idt", [128, 128], F32R, isOutput=False)
    on64 = nc.declare_dram_parameter("on64", [1, 64], F32R, isOutput=False)
    on128 = nc.declare_dram_parameter("on128", [128, 1], F32R, isOutput=False)
    out = nc.declare_dram_parameter("out", [TPC, D], F32, isOutput=True)

    # internal DRAM
    apart = [nc.dram_tensor(f"apart{rep}", [NTT, NCORES * 128, D], F32)
             for rep in range(n_rep)]
    rso = [nc.dram_tensor(f"rso{rep}", [NTT, 128, D], F32) for rep in range(n_rep)]

    with TileContext(nc) as tc:
        with tc.tile_pool(name="poolA", bufs=1) as pA, \
             tc.tile_pool(name="psum", bufs=1, space="PSUM") as pp:

            # ---- persistent small tiles ----
            wq_s = pA.tile([128, KT, 128], F32R, tag="wq")
            wk_s = pA.tile([128, KT, 128], F32R, tag="wk")
            wv_s = pA.tile([128, KT, 128], F32R, tag="wv")
            bq_s = pA.tile([128, 1], F32, tag="bq")
            bk_s = pA.tile([128, 1], F32, tag="bk")
            bv_s = pA.tile([128, 1], F32, tag="bv")
            wo_s = pA.tile([128, D], F32R, tag="wo")
            b1_s = pA.tile([128, FT], F32, tag="b1")
            b2_s = pA.tile([128, D], F32, tag="b2")
            bo_s = pA.tile([128, D], F32, tag="bo")
            l1g_s = pA.tile([128, D], F32, tag="l1g")
            l1b_s = pA.tile([128, D], F32, tag="l1b")
            l2g_s = pA.tile([128, D], F32, tag="l2g")
            l2b_s = pA.tile([128, D], F32, tag="l2b")
            msk_s = pA.tile([128, 4, 512], F32, tag="msk")
            idt_s = pA.tile([128, 128], F32R, tag="idt")
            on_s = pA.tile([1, 64], F32R, tag="on64")
            on1_s = pA.tile([128, 1], F32R, tag="on128")
            eps_s = pA.tile([128, 1], F32, tag="eps")
            nc.vector.memset(eps_s[:, :], EPS)
            for dst, src in [(wq_s, wq), (wk_s, wk), (wv_s, wv),
                             (bq_s, bq), (bk_s, bk), (bv_s, bv), (wo_s, wo),
                             (b1_s, b1s), (b2_s, b2b), (bo_s, bob),
                             (l1g_s, l1g), (l1b_s, l1b), (l2g_s, l2g),
                             (l2b_s, l2b), (idt_s, idt), (on_s, on64), (on1_s, on128)]:
                nc.sync.dma_start(out=dst[...], in_=src[...])
            nc.sync.dma_start(out=msk_s[:, :, :], in_=msk[:, :, :])

            for rep in range(n_rep):
                emit_once(nc, tc, pA, pp, rep, apart[rep], rso[rep],
                          xt, xfm, out, skip, stop_after, attn_mode,
                          wq_s, wk_s, wv_s, bq_s, bk_s, bv_s, wo_s,
                          w1, b1_s, w2, b2_s, bo_s,
                          l1g_s, l1b_s, l2g_s, l2b_s, msk_s, idt_s, on_s,
                          on1_s, eps_s, AF)
    nc.compile()
    return nc


def layer_norm_inplace(nc, pC, x_t, g_s, b_s, eps_s, AF, tag, final_f32r=False):
    """LN over the free dim (1024) of x_t (128, 1024), in place."""
    st = pC.tile([128, 1], F32, tag=f"st{tag}", name=f"st{tag}")
    nc.vector.tensor_reduce(st[:, :], x_t[:, :], mybir.AxisListType.X,
                            mybir.AluOpType.add)
    nmu = pC.tile([128, 1], F32, tag=f"nmu{tag}", name=f"nmu{tag}")
    nc.scalar.activation(nmu[:, :], st[:, :], AF.Copy, scale=-1.0 / D)
    nc.vector.tensor_scalar_add(x_t[:, :], x_t[:, :], nmu[:, :])
    sq = pC.tile([128, D], F32, tag=f"sq{tag}", name=f"sq{tag}")
    vs = pC.tile([128, 1], F32, tag=f"vs{tag}", name=f"vs{tag}")
    nc.scalar.activation(sq[:, :], x_t[:, :], AF.Square, accum_out=vs[:, :])
    sd = pC.tile([128, 1], F32, tag=f"sd{tag}", name=f"sd{tag}")
    nc.scalar.activation(sd[:, :], vs[:, :], AF.Sqrt, scale=1.0 / D,
                         bias=eps_s[:, :])
    rstd = pC.tile([128, 1], F32, tag=f"rstd{tag}", name=f"rstd{tag}")
    nc.vector.reciprocal(rstd[:, :], sd[:, :])
    nc.vector.tensor_scalar_mul(x_t[:, :], x_t[:, :], rstd[:, :])
    nc.vector.tensor_mul(out=x_t[:, :], in0=x_t[:, :], in1=g_s[:, :])
    last_out = r(x_t[:, :]) if final_f32r else x_t[:, :]
    nc.vector.tensor_add(out=last_out, in0=x_t[:, :], in1=b_s[:, :])



def _anchor(nc, pB, out, tiles, col):
    """Consume tiles cheaply: reduce each to (P,1) and DMA into out column."""
    for j, t in enumerate(tiles):
        a = pB.tile([t.shape[0], 1], F32, tag="anch", bufs=2, name=f"anch_{col}_{j}")
        nc.vector.tensor_reduce(a[:, :], t[...].bitcast(F32), mybir.AxisListType.X,
                                mybir.AluOpType.add)
        nc.sync.dma_start(out=out[j % 4 * 128:j % 4 * 128 + t.shape[0],
                                  col + j // 4:col + j // 4 + 1], in_=a[:, :])

def emit_once(nc, tc, pA, pp, rep, apart, rso, xt, xfm, out, skip, stop_after,
              attn_mode, wq_s, wk_s, wv_s, bq_s, bk_s, bv_s, wo_s,
              w1, b1_s, w2, b2_s, bo_s,
              l1g_s, l1b_s, l2g_s, l2b_s, msk_s, idt_s, on_s, on1_s, eps_s, AF):
    RG = [list(range(NCORES))]

    with tc.tile_pool(name=f"poolB{rep}", bufs=1) as pB:
        # ctx2[b][qc]: (128 d, 512 q) normalized context, both heads stacked
        ctx2 = {}
        if attn_mode is None:
            for b in range(B):
                for qc in range(QC):
                    ctx2[(b, qc)] = pA.tile([128, 512], F32R, tag=f"c2_{b}_{qc}",
                                            name=f"c2_{b}_{qc}")

        for b in range(B):
            if "attn" in skip:
                break
            qt_s = pB.tile([128, T], F32R, tag="qt")
            kt_s = pB.tile([128, T], F32R, tag="kt")
            vt_s = pB.tile([128, T], F32R, tag="vt")
            # ---- QKV projections for this batch ----
            for tc_ in range(QC):
                xts = pB.tile([128, KT, 512], F32R, tag="xts", bufs=2)
                nc.sync.dma_start(out=xts[:, :, :],
                                  in_=xt[b, :, :, tc_ * 512:(tc_ + 1) * 512])
                for (w_s, b_s, dst) in ((wq_s, bq_s, qt_s), (wk_s, bk_s, kt_s),
                                        (wv_s, bv_s, vt_s)):
                    ps = pp.tile([128, 512], F32, tag="mm", bufs=3, name="ps")
                    for k in range(KT):
                        nc.tensor.matmul(ps[:, :], w_s[:, k, :], xts[:, k, :],
                                         start=(k == 0), stop=(k == KT - 1))
                    nc.scalar.activation(dst[:, tc_ * 512:(tc_ + 1) * 512],
                                         ps[:, :], AF.Identity, bias=b_s[:, :])

            # ---- V transpose: vT (128 dims, T) -> v2 tiles (128 tok, 130) ----
            v2 = []
            for kt in range(TT):
                tp = pp.tile([128, 128], F32R, tag="mm", bufs=3, name=f"tp_{kt}")
                nc.tensor.transpose(tp[:, :], vt_s[:, kt * 128:(kt + 1) * 128],
                                    idt_s[:, :])
                v2t = pB.tile([128, 130], F32R, tag=f"v2_{kt}", name=f"v2_{kt}")
                nc.vector.tensor_copy(v2t[:, 64:65], on1_s[:, :])
                nc.vector.tensor_copy(v2t[:, 129:130], on1_s[:, :])
                nc.vector.tensor_copy(v2t[:, 0:64], tp[:, 0:64])
                nc.vector.tensor_copy(v2t[:, 65:129], tp[:, 64:128])
                v2.append(v2t)

            if stop_after == "qkv":
                _anchor(nc, pB, out, [qt_s, kt_s] + v2, 100 * (b + 1))
                continue
            # ---- causal attention, per head ----
            for h in range(2):
                hs = slice(h * 64, (h + 1) * 64)
                vs_lo, vs_hi = (0, 65) if h == 0 else (65, 130)
                for qc in range(QC):
                    ctxp = pp.tile([65, 512], F32, tag="ctx", bufs=1, name="ctxp")
                    n_kt = 4 * qc + 4
                    prev_p = None
                    for kt in range(n_kt):
                        sp = pp.tile([128, 512], F32, tag="mm", bufs=3, name="sp")
                        nc.tensor.matmul(
                            sp[:, :],
                            kt_s[hs, kt * 128:(kt + 1) * 128],
                            qt_s[hs, qc * 512:(qc + 1) * 512],
                            start=True, stop=True)
                        if attn_mode == "qk":
                            a = pB.tile([128, 1], F32, tag="anch", bufs=2, name="aq")
                            nc.vector.tensor_reduce(a[:, :], sp[:, :],
                                                    mybir.AxisListType.X,
                                                    mybir.AluOpType.add)
                            nc.sync.dma_start(out=out[(kt % 4) * 128:(kt % 4 + 1) * 128,
                                                      qc:qc + 1], in_=a[:, :])
                            continue
                        if kt >= 4 * qc and attn_mode != "qkexp":
                            nc.vector.tensor_add(out=sp[:, :], in0=sp[:, :],
                                                 in1=msk_s[:, kt % 4, :])
                        p = pB.tile([128, 512], F32R, tag="p", bufs=6, name="p")
                        nc.scalar.activation(p[:, :], sp[:, :], AF.Exp,
                                             scale=0.125)
                        if attn_mode in ("qkexp", "qkexpmask"):
                            a = pB.tile([128, 1], F32, tag="anch", bufs=2, name="aq")
                            nc.vector.tensor_reduce(a[:, :], p[:, :].bitcast(F32),
                                                    mybir.AxisListType.X,
                                                    mybir.AluOpType.add)
                            nc.sync.dma_start(out=out[(kt % 4) * 128:(kt % 4 + 1) * 128,
                                                      qc:qc + 1], in_=a[:, :])
                            continue
                        if prev_p is not None:
                            pkt, pt_ = prev_p
                            nc.tensor.matmul(ctxp[:, :], v2[pkt][:, vs_lo:vs_hi],
                                             pt_[:, :],
                                             start=(pkt == 0), stop=False)
                        prev_p = (kt, p)
                    if attn_mode in ("qk", "qkexp", "qkexpmask"):
                        continue
                    pkt, pt_ = prev_p
                    nc.tensor.matmul(ctxp[:, :], v2[pkt][:, vs_lo:vs_hi],
                                     pt_[:, :], start=(pkt == 0), stop=True)
                    # normalize: rows 0..63 ctx, row 64 denominator
                    rec = pB.tile([1, 512], F32R, tag="rec", bufs=2, name="rec")
                    with nc.allow_low_precision(reason="f32r is full precision"):
                        nc.vector.reciprocal(rec[:, :], ctxp[64:65, :])
                    bc = pp.tile([64, 512], F32, tag="mm", bufs=3, name="bc")
                    nc.tensor.matmul(bc[:, :], on_s[:, :], rec[:, :],
                                     start=True, stop=True)
                    bcs = pB.tile([64, 512], F32, tag="bcs", bufs=2, name="bcs")
                    nc.scalar.activation(bcs[:, :], bc[:, :], AF.Copy)
                    nc.vector.tensor_mul(out=ctx2[(b, qc)][hs.start:hs.stop, :],
                                         in0=ctxp[0:64, :], in1=bcs[:, :])

        if stop_after == "qkv":
            return
        if stop_after == "attn":
            _anchor(nc, pB, out, list(ctx2.values()), 300)
            return
        # ---- W_o partials + chunked ReduceScatter ----
        for tt in range(NTT if "wo" not in skip else 0):
            for b in range(B):
                for qc in range(QC):
                    ps = pp.tile([128, D], F32, tag="f2a", bufs=1,
                                 name=f"wops_{tt}_{b}_{qc}")
                    for oc in range(2):
                        nc.tensor.matmul(
                            ps[:, oc * 512:(oc + 1) * 512],
                            ctx2[(b, qc)][:, tt * 128:(tt + 1) * 128],
                            wo_s[:, oc * 512:(oc + 1) * 512],
                            start=True, stop=True)
                    ao = pB.tile([128, D], F32, tag="ao", bufs=3, name="ao")
                    nc.scalar.activation(ao[:, :], ps[:, :], AF.Copy)
                    core = 4 * b + qc
                    nc.sync.dma_start(
                        out=apart[tt, core * 128:(core + 1) * 128, :],
                        in_=ao[:, :])
            if "rs" not in skip:
                nc.gpsimd.collective_compute(
                    "ReduceScatter", mybir.AluOpType.add, replica_groups=RG,
                    ins=[apart[tt]], outs=[rso[tt]])

        if stop_after == "rs":
            for tt in range(NTT):
                nc.sync.dma_start(out=out[tt * 128:(tt + 1) * 128, :],
                                  in_=rso[tt, :, :])
            return
    # ---- FFN phase ----
    if "ffn" in skip:
        return
    with tc.tile_pool(name=f"poolC{rep}", bufs=1) as pC:
        x1t_s = pC.tile([128, KT, TPC], F32R, tag="x1t")
        x1 = []
        for tt in range(NTT):
            xr = pC.tile([128, D], F32, tag="xr", bufs=1, name="xr")
            nc.sync.dma_start(out=xr[:, :], in_=xfm[tt * 128:(tt + 1) * 128, :])
            asb = pC.tile([128, D], F32, tag="asb", bufs=1, name="asb")
            nc.sync.dma_start(out=asb[:, :], in_=rso[tt, :, :])
            x1_tt = pA.tile([128, D], F32, tag=f"x1_{tt}", name=f"x1_{tt}")
            nc.vector.tensor_add(out=x1_tt[:, :], in0=asb[:, :], in1=bo_s[:, :])
            nc.vector.tensor_add(out=x1_tt[:, :], in0=x1_tt[:, :], in1=xr[:, :])
            layer_norm_inplace(nc, pC, x1_tt, l1g_s, l1b_s, eps_s, AF, "a")
            x1.append(x1_tt)
            x1c = pC.tile([128, D], F32R, tag="x1c", bufs=2, name="x1c")
            nc.vector.tensor_copy(x1c[:, :], x1_tt[:, :])
            for i in range(KT):
                tp2 = pp.tile([128, 128], F32R, tag="mm", bufs=3,
                              name=f"tp2_{tt}_{i}")
                nc.tensor.transpose(tp2[:, :], x1c[:, i * 128:(i + 1) * 128],
                                    idt_s[:, :])
                nc.vector.tensor_copy(x1t_s[:, i, tt * 128:(tt + 1) * 128],
                                      tp2[:, :])

        if stop_after == "ln1":
            _anchor(nc, pC, out, x1 + [x1t_s], 300)
            return
        # FFN1: hT[ft] (128 f, 512 t)
        hts = []
        for ft in range(FT):
            w1s = pC.tile([128, KT, 128], F32R, tag="w1s", bufs=2, name="w1s")
            nc.sync.dma_start(out=w1s[:, :, :],
                              in_=w1[:, :, ft * 128:(ft + 1) * 128])
            ps = pp.tile([128, 512], F32, tag="mm", bufs=3, name=f"f1ps_{ft}")
            for k in range(KT):
                nc.tensor.matmul(ps[:, :], w1s[:, k, :], x1t_s[:, k, :],
                                 start=(k == 0), stop=(k == KT - 1))
            h_ft = pC.tile([128, TPC], F32R, tag=f"h_{ft}", name=f"h_{ft}")
            nc.scalar.activation(h_ft[:, :], ps[:, :], AF.Relu,
                                 bias=b1_s[:, ft:ft + 1])
            hts.append(h_ft)

        if stop_after == "ffn1":
            _anchor(nc, pC, out, hts, 300)
            return
        # FFN2 + LN2, token tiles in pairs
        for pair in range(NTT // 2):
            tts = (2 * pair, 2 * pair + 1)
            pss = {tts[0]: pp.tile([128, D], F32, tag="f2a", bufs=1,
                                   name=f"f2a_{pair}"),
                   tts[1]: pp.tile([128, D], F32, tag="f2b", bufs=1,
                                   name=f"f2b_{pair}")}
            for kf in range(FT):
                w2s = pC.tile([128, D], F32R, tag="w2s", bufs=2, name="w2s")
                nc.sync.dma_start(out=w2s[:, :], in_=w2[:, kf, :])
                for tt in tts:
                    for oc in range(2):
                        nc.tensor.matmul(
                            pss[tt][:, oc * 512:(oc + 1) * 512],
                            hts[kf][:, tt * 128:(tt + 1) * 128],
                            w2s[:, oc * 512:(oc + 1) * 512],
                            start=(kf == 0), stop=(kf == FT - 1))
            for tt in tts:
                y = pC.tile([128, D], F32, tag="y", bufs=1, name="y")
                nc.vector.tensor_add(out=y[:, :], in0=pss[tt][:, :],
                                     in1=b2_s[:, :])
                nc.vector.tensor_add(out=y[:, :], in0=y[:, :], in1=x1[tt][:, :])
                layer_norm_inplace(nc, pC, y, l2g_s, l2b_s, eps_s, AF, "b")
                nc.sync.dma_start(out=out[tt * 128:(tt + 1) * 128, :],
                                  in_=y[:, :])


# ---------------- host side ----------------

_CACHE = {}


def _get_kernel(n_rep=1, skip=(), stop_after=None, attn_mode=None):
    key = (n_rep, tuple(sorted(skip)), stop_after, attn_mode)
    if key not in _CACHE:
        _CACHE[key] = build_kernel(n_rep, skip, stop_after, attn_mode)
    return _CACHE[key]


def make_in_maps(x, Wq, bq, Wk, bk, Wv, bv, Wo, bo, W1, b1, W2, b2,
                 ln1_g, ln1_b, ln2_g, ln2_b):
    x = np.asarray(x, np.float32)
    xf = x.reshape(B * T, D)
    # xt[b, p, k, t] = x[b, t, 128k+p]
    xtv = np.ascontiguousarray(
        x.reshape(B, T, KT, 128).transpose(0, 3, 2, 1))
    w1t = np.ascontiguousarray(
        np.asarray(W1, np.float32).T.reshape(KT, 128, D_FF).transpose(1, 0, 2))
    w2t = np.ascontiguousarray(
        np.asarray(W2, np.float32).T.reshape(FT, 128, D).transpose(1, 0, 2))
    b1sv = np.ascontiguousarray(np.asarray(b1, np.float32).reshape(FT, 128).T)
    ones = np.ones((128, 1), np.float32)
    b2bv = ones @ np.asarray(b2, np.float32)[None, :]
    bobv = ones @ np.asarray(bo, np.float32)[None, :]
    l1gb = ones @ np.asarray(ln1_g, np.float32)[None, :]
    l1bb = ones @ np.asarray(ln1_b, np.float32)[None, :]
    l2gb = ones @ np.asarray(ln2_g, np.float32)[None, :]
    l2bb = ones @ np.asarray(ln2_b, np.float32)[None, :]
    # additive causal masks for diagonal blocks, r = kt%4; layout (128,4,512)
    kk = np.arange(128)[:, None]
    qq = np.arange(512)[None, :]
    mskv = np.ascontiguousarray(np.stack(
        [np.where(qq >= r * 128 + kk, 0.0, NEG).astype(np.float32)
         for r in range(4)]).transpose(1, 0, 2))
    idtv = np.eye(128, dtype=np.float32)
    on64v = np.ones((1, 64), np.float32)

    in_maps = []
    for c in range(NCORES):
        sl = slice(128 * c, 128 * (c + 1))

        def wslice(W):
            # [128 p(in-dim), KT, 128 m(out-dim)]: w[p,k,m]=W[128c+m, 128k+p]
            Ws = np.asarray(W, np.float32)[sl, :]  # (128 out, 1024 in)
            return np.ascontiguousarray(
                Ws.T.reshape(KT, 128, 128).transpose(1, 0, 2))

        m = dict(
            xt=xtv, xfm=np.ascontiguousarray(xf[c * TPC:(c + 1) * TPC]),
            wq=wslice(Wq), wk=wslice(Wk), wv=wslice(Wv),
            bq=np.asarray(bq, np.float32)[sl][:, None].copy(),
            bk=np.asarray(bk, np.float32)[sl][:, None].copy(),
            bv=np.asarray(bv, np.float32)[sl][:, None].copy(),
            wo=np.ascontiguousarray(np.asarray(Wo, np.float32)[:, sl].T),
            w1=w1t, b1s=b1sv, w2=w2t, b2b=b2bv, bob=bobv,
            l1g=l1gb, l1b=l1bb, l2g=l2gb, l2b=l2bb,
            msk=mskv, idt=idtv, on64=on64v, on128=ones,
        )
        in_maps.append(m)
    return in_maps


def kernel(x, mask, Wq, bq, Wk, bk, Wv, bv, Wo, bo, W1, b1, W2, b2,
           ln1_g, ln1_b, ln2_g, ln2_b):
    del mask  # causal mask is hardcoded in the device program
    nc = _get_kernel(1)
    in_maps = make_in_maps(x, Wq, bq, Wk, bk, Wv, bv, Wo, bo, W1, b1, W2, b2,
                           ln1_g, ln1_b, ln2_g, ln2_b)
    res = run_bass_kernel_spmd(nc, in_maps, list(range(NCORES)))
    full = np.concatenate([res.results[c]["out"] for c in range(NCORES)], axis=0)
    return full.reshape(B, T, D).astype(np.float32)
